# revision 16
# baseline (speedup 1.0000x reference)
"""Trainium2 Bass kernel: DiscreteEmbedding (rect-window embedding lookup).

Math (matches the jax reference up to fp16 table rounding):
    xs  = x * 2048;  y = xs + 0.5
    i_lo = floor(y)  (exact for non-boundary tokens)
    boundary tokens (y integer): out = avg of rows y-1, y -> combined table
Combined table TC (built on the HOST in fp16):
    TC[0:2048]  = T        (plain rows; TC[2048] = 0 zero row)
    TC[2048+k]  = (T[k-1]+T[k])/2 for k>=1   i.e. idx = i0 + 2048*b,
    b = (y integer), i0 = trunc(y).
Device: per-token SWDGE gather from TC, no device-side table prep.

Strategy notes:
  - fp16 halves all table/gather/store bytes; rel err ~2e-4.
  - Index math is sliced into 4 slabs of 2048 tokens so the first gather
    chunk dispatches as soon as slab 0 is done (SWDGE desc-gen at ~10ns/idx
    per queue-pair is the serial floor; start it early, keep 4 queues fed).
  - x loads split across both HWDGE rings (sync+scalar) to halve latency.
  - 8 uniform 1024-idx gather chunks: single num_idxs register (no $R WAR
    stall on the in-order GPSIMD engine), 12 SWDGE insts over 8 DMASW sem
    lanes (no gather-waits-gather ring serialization).
  - Gather output is position-ordered; stores alternate the two HWDGE
    rings; host un-permutes rows + casts fp16->f32 while un-sharding.
"""

import numpy as np

import concourse.mybir as mybir
import concourse.tile as tile
from concourse import bacc, bass_utils

N_CORES = 8
B, S = 32, 2048
V, D = 2048, 128
TOK = B * S                 # 65536 tokens total
TPC = TOK // N_CORES        # 8192 tokens per core
SPC = TPC // 16             # 512: free dim of the wrapped [16, 512] x layout
VEXT = 4224                 # TC rows (>= 2*V+1, multiple of 128)
NQ = 4                      # SWDGE queues
# gather chunks (num_idxs, queue): small round-1 chunks so all 4 queue
# pairs spin up quickly (a dispatch blocks the in-order GPSIMD stream
# for its own desc-gen), then big round-2 chunks.
CHUNKS = [(256, 0), (256, 1), (256, 2), (256, 3),
          (1792, 0), (1792, 1), (1792, 2), (1792, 3)]

F32 = mybir.dt.float32
F16 = mybir.dt.float16
I32 = mybir.dt.int32
I16 = mybir.dt.int16
OP = mybir.AluOpType


def build():
    nc = bacc.Bacc(
        "TRN2",
        target_bir_lowering=False,
        debug=False,
        num_devices=N_CORES,
        num_swdge_queues=NQ,
    )
    xr = nc.dram_tensor("xr", [128, SPC], F32, kind="ExternalInput")
    tc_tbl = nc.dram_tensor("tc_tbl", [VEXT, D], F16, kind="ExternalInput")
    out = nc.dram_tensor("out", [TPC, D], F16, kind="ExternalOutput")

    with tile.TileContext(nc) as tc:
        with tc.tile_pool(name="sb", bufs=1) as sb, tc.tile_pool(name="g", bufs=1) as gp:
            # ---- all gpsimd memsets + reg loads first ----
            zidx = sb.tile([128, 16], I16)
            nc.gpsimd.memset(zidx[:], 0)
            reg16 = nc.gpsimd.to_reg(16)
            nidx_regs = {
                n: nc.gpsimd.to_reg(n) for n in sorted({c[0] for c in CHUNKS})
            }
            # ---- warm-ups: pay the Q7 library-load + per-pair ring init
            # before the first real chunks.
            with tc.high_priority():
                for q in range(NQ):
                    wg = sb.tile([128, D], F16, tag=f"warm{q}")
                    nc.gpsimd.dma_gather(
                        wg[:].rearrange("p (j d) -> p j d", d=D),
                        tc_tbl[:],
                        zidx[:, 0:1],
                        num_idxs=16,
                        num_idxs_reg=reg16,
                        elem_size=D,
                        single_packet=False,
                        queue_num=q,
                    )

            # x halves on both HWDGE rings in parallel
            xt = sb.tile([128, SPC], F32)
            nc.sync.dma_start(out=xt[:, 0 : SPC // 2], in_=xr[:, 0 : SPC // 2])
            nc.scalar.dma_start(out=xt[:, SPC // 2 :], in_=xr[:, SPC // 2 :])

            # ---- index math: one full-width pass (the first SWDGE op is
            # gated on the end of the vector program, so ending the chain
            # sooner beats slab-chunking it) ----
            y = sb.tile([128, SPC], F32)
            nc.vector.tensor_scalar(y[:], xt[:], 2048.0, 0.5, op0=OP.mult, op1=OP.add)
            i0 = sb.tile([128, SPC], I32)
            nc.vector.tensor_copy(i0[:], y[:])       # RNE: floor or ceil
            f0 = sb.tile([128, SPC], F32)
            nc.vector.tensor_copy(f0[:], i0[:])
            gt = sb.tile([128, SPC], F32)
            nc.vector.tensor_tensor(gt[:], f0[:], y[:], op=OP.is_gt)
            bnd = sb.tile([128, SPC], F32)
            nc.vector.tensor_tensor(bnd[:], f0[:], y[:], op=OP.is_equal)
            # idx = floor(y) + 2048*b = f0 - gt + 2048*b
            u = sb.tile([128, SPC], F32)
            nc.vector.scalar_tensor_tensor(
                out=u[:], in0=bnd[:], scalar=float(V), in1=gt[:],
                op0=OP.mult, op1=OP.subtract,
            )
            idxf = sb.tile([128, SPC], F32)
            nc.vector.tensor_add(idxf[:], u[:], f0[:])
            idx16 = sb.tile([128, SPC], I16)
            nc.vector.tensor_copy(idx16[:], idxf[:])

            # ---- chunked gather + store ----
            out_v = out[:].rearrange("(p j) d -> p (j d)", p=128)
            pos = 0
            for ci, (nix, q) in enumerate(CHUNKS):
                jb = nix // 128
                g = gp.tile([128, jb * D], F16, tag=f"g{ci}")
                nc.gpsimd.dma_gather(
                    g[:].rearrange("p (j d) -> p j d", d=D),
                    tc_tbl[:],
                    idx16[:, pos // 16 : (pos + nix) // 16],
                    num_idxs=nix,
                    num_idxs_reg=nidx_regs[nix],
                    elem_size=D,
                    single_packet=False,
                    queue_num=q,
                )
                st_eng = nc.sync if ci % 2 == 0 else nc.scalar
                st_eng.dma_start(
                    out=out_v[:, (pos // 128) * D : ((pos + nix) // 128) * D], in_=g[:]
                )
                pos += nix
            assert pos == TPC
    nc.compile()
    return nc


_NC = None


def _row_perm():
    """out row r holds gather position i(r); position i handles token
    t(i) = (i%16)*512 + i//16 (x wrapped [16,512] across partitions)."""
    r = np.arange(TPC)
    p, j = r // 64, r % 64
    i = j * 128 + p
    return (i % 16) * SPC + i // 16  # token index held at row r


def _build_tc(t):
    """Host-side combined table, fp16: [T; 0 at 2048; avg-pairs at 2048+k]."""
    tc = np.zeros((VEXT, D), dtype=np.float32)
    tc[0:V] = t
    tz = np.vstack([t, np.zeros((1, D), np.float32)])
    # boundary y=k integer (k in 1..2048): avg of rows k-1, k at index 2048+k
    tc[V + 1 : V + 1 + V] = 0.5 * (tz[0:V] + tz[1 : V + 1])
    return tc.astype(np.float16)


def kernel(x, time_embedding):
    global _NC
    x = np.ascontiguousarray(np.asarray(x, dtype=np.float32))
    t = np.ascontiguousarray(np.asarray(time_embedding, dtype=np.float32))
    tc16 = _build_tc(t)
    xf = x.reshape(-1)
    in_maps = []
    for c in range(N_CORES):
        xc = xf[c * TPC : (c + 1) * TPC].reshape(16, SPC)
        in_maps.append({"xr": np.ascontiguousarray(np.tile(xc, (8, 1))), "tc_tbl": tc16})

    if _NC is None:
        _NC = build()
    res = bass_utils.run_bass_kernel_spmd(_NC, in_maps, core_ids=list(range(N_CORES)))
    global _LAST_RES
    _LAST_RES = res

    tkn = _row_perm()
    outs = []
    for c in range(N_CORES):
        oc = np.asarray(res.results[c]["out"]).astype(np.float32)
        full = np.empty_like(oc)
        full[tkn] = oc
        outs.append(full)
    return np.concatenate(outs, axis=0).reshape(B, S, D)


# revision 18
# speedup vs baseline: 1.0841x; 1.0841x over previous
"""Trainium2 Bass kernel: DiscreteEmbedding (rect-window embedding lookup).

Math (matches the jax reference up to fp16 table rounding):
    xs  = x * 2048;  y = xs + 0.5
    i_lo = floor(y)  (exact for non-boundary tokens)
    boundary tokens (y integer): out = avg of rows y-1, y -> combined table
Combined table TC (built on the HOST in fp16):
    TC[0:2048]  = T        (plain rows; TC[2048] = 0 zero row)
    TC[2048+k]  = (T[k-1]+T[k])/2 for k>=1   i.e. idx = i0 + 2048*b,
    b = (y integer), i0 = trunc(y).
Device: per-token SWDGE gather from TC, no device-side table prep.

Strategy notes:
  - fp16 halves all table/gather/store bytes; rel err ~2e-4.
  - Index math is sliced into 4 slabs of 2048 tokens so the first gather
    chunk dispatches as soon as slab 0 is done (SWDGE desc-gen at ~10ns/idx
    per queue-pair is the serial floor; start it early, keep 4 queues fed).
  - x loads split across both HWDGE rings (sync+scalar) to halve latency.
  - 8 uniform 1024-idx gather chunks: single num_idxs register (no $R WAR
    stall on the in-order GPSIMD engine), 12 SWDGE insts over 8 DMASW sem
    lanes (no gather-waits-gather ring serialization).
  - Gather output is position-ordered; stores alternate the two HWDGE
    rings; host un-permutes rows + casts fp16->f32 while un-sharding.
"""

import numpy as np

import concourse.mybir as mybir
import concourse.tile as tile
from concourse import bacc, bass_utils

N_CORES = 8
B, S = 32, 2048
V, D = 2048, 128
TOK = B * S                 # 65536 tokens total
TPC = TOK // N_CORES        # 8192 tokens per core
SPC = TPC // 16             # 512: free dim of the wrapped [16, 512] x layout
VEXT = 4224                 # TC rows (>= 2*V+1, multiple of 128)
NQ = 4                      # SWDGE queues
# gather chunks (num_idxs, queue): uniform 1024-idx chunks round-robin
CHUNKS = [(1024, 0), (1024, 1), (1024, 2), (1024, 3),
          (1024, 0), (1024, 1), (1024, 2), (1024, 3)]

F32 = mybir.dt.float32
F16 = mybir.dt.float16
I32 = mybir.dt.int32
I16 = mybir.dt.int16
OP = mybir.AluOpType


def build():
    nc = bacc.Bacc(
        "TRN2",
        target_bir_lowering=False,
        debug=False,
        num_devices=N_CORES,
        num_swdge_queues=NQ,
    )
    xr = nc.dram_tensor("xr", [128, SPC], F32, kind="ExternalInput")
    tc_tbl = nc.dram_tensor("tc_tbl", [VEXT, D], F16, kind="ExternalInput")
    out = nc.dram_tensor("out", [TPC, D], F16, kind="ExternalOutput")

    with tile.TileContext(nc) as tc:
        with tc.tile_pool(name="sb", bufs=1) as sb, tc.tile_pool(name="g", bufs=1) as gp:
            # ---- all gpsimd memsets + reg loads first ----
            zidx = sb.tile([128, 16], I16)
            nc.gpsimd.memset(zidx[:], 0)
            reg16 = nc.gpsimd.to_reg(16)
            nidx_regs = {
                n: nc.gpsimd.to_reg(n) for n in sorted({c[0] for c in CHUNKS})
            }
            # ---- single warm-up on queue 3: pays the Q7 library-load +
            # ring init without colliding with round-1 chunk dispatches
            # (q3 gets its first real chunk last).
            with tc.high_priority():
                wg = sb.tile([128, D], F16, tag="warm0")
                nc.gpsimd.dma_gather(
                    wg[:].rearrange("p (j d) -> p j d", d=D),
                    tc_tbl[:],
                    zidx[:, 0:1],
                    num_idxs=16,
                    num_idxs_reg=reg16,
                    elem_size=D,
                    single_packet=False,
                    queue_num=3,
                )

            # x halves on both HWDGE rings in parallel
            xt = sb.tile([128, SPC], F32)
            nc.sync.dma_start(out=xt[:, 0 : SPC // 2], in_=xr[:, 0 : SPC // 2])
            nc.scalar.dma_start(out=xt[:, SPC // 2 :], in_=xr[:, SPC // 2 :])

            # ---- index math: one full-width pass (the first SWDGE op is
            # gated on the end of the vector program, so ending the chain
            # sooner beats slab-chunking it) ----
            y = sb.tile([128, SPC], F32)
            nc.vector.tensor_scalar(y[:], xt[:], 2048.0, 0.5, op0=OP.mult, op1=OP.add)
            i0 = sb.tile([128, SPC], I32)
            nc.vector.tensor_copy(i0[:], y[:])       # RNE: floor or ceil
            f0 = sb.tile([128, SPC], F32)
            nc.vector.tensor_copy(f0[:], i0[:])
            gt = sb.tile([128, SPC], F32)
            nc.vector.tensor_tensor(gt[:], f0[:], y[:], op=OP.is_gt)
            bnd = sb.tile([128, SPC], F32)
            nc.vector.tensor_tensor(bnd[:], f0[:], y[:], op=OP.is_equal)
            # idx = floor(y) + 2048*b = f0 - gt + 2048*b
            u = sb.tile([128, SPC], F32)
            nc.vector.scalar_tensor_tensor(
                out=u[:], in0=bnd[:], scalar=float(V), in1=gt[:],
                op0=OP.mult, op1=OP.subtract,
            )
            idxf = sb.tile([128, SPC], F32)
            nc.vector.tensor_add(idxf[:], u[:], f0[:])
            idx16 = sb.tile([128, SPC], I16)
            nc.vector.tensor_copy(idx16[:], idxf[:])

            # ---- chunked gather + store ----
            out_v = out[:].rearrange("(p j) d -> p (j d)", p=128)
            pos = 0
            for ci, (nix, q) in enumerate(CHUNKS):
                jb = nix // 128
                g = gp.tile([128, jb * D], F16, tag=f"g{ci}")
                nc.gpsimd.dma_gather(
                    g[:].rearrange("p (j d) -> p j d", d=D),
                    tc_tbl[:],
                    idx16[:, pos // 16 : (pos + nix) // 16],
                    num_idxs=nix,
                    num_idxs_reg=nidx_regs[nix],
                    elem_size=D,
                    single_packet=False,
                    queue_num=q,
                )
                st_eng = nc.sync if ci % 2 == 0 else nc.scalar
                st_eng.dma_start(
                    out=out_v[:, (pos // 128) * D : ((pos + nix) // 128) * D], in_=g[:]
                )
                pos += nix
            assert pos == TPC
    nc.compile()
    return nc


_NC = None


def _row_perm():
    """out row r holds gather position i(r); position i handles token
    t(i) = (i%16)*512 + i//16 (x wrapped [16,512] across partitions)."""
    r = np.arange(TPC)
    p, j = r // 64, r % 64
    i = j * 128 + p
    return (i % 16) * SPC + i // 16  # token index held at row r


def _build_tc(t):
    """Host-side combined table, fp16: [T; 0 at 2048; avg-pairs at 2048+k]."""
    tc = np.zeros((VEXT, D), dtype=np.float32)
    tc[0:V] = t
    tz = np.vstack([t, np.zeros((1, D), np.float32)])
    # boundary y=k integer (k in 1..2048): avg of rows k-1, k at index 2048+k
    tc[V + 1 : V + 1 + V] = 0.5 * (tz[0:V] + tz[1 : V + 1])
    return tc.astype(np.float16)


def kernel(x, time_embedding):
    global _NC
    x = np.ascontiguousarray(np.asarray(x, dtype=np.float32))
    t = np.ascontiguousarray(np.asarray(time_embedding, dtype=np.float32))
    tc16 = _build_tc(t)
    xf = x.reshape(-1)
    in_maps = []
    for c in range(N_CORES):
        xc = xf[c * TPC : (c + 1) * TPC].reshape(16, SPC)
        in_maps.append({"xr": np.ascontiguousarray(np.tile(xc, (8, 1))), "tc_tbl": tc16})

    if _NC is None:
        _NC = build()
    res = bass_utils.run_bass_kernel_spmd(_NC, in_maps, core_ids=list(range(N_CORES)))
    global _LAST_RES
    _LAST_RES = res

    tkn = _row_perm()
    outs = []
    for c in range(N_CORES):
        oc = np.asarray(res.results[c]["out"]).astype(np.float32)
        full = np.empty_like(oc)
        full[tkn] = oc
        outs.append(full)
    return np.concatenate(outs, axis=0).reshape(B, S, D)


# revision 20
# speedup vs baseline: 1.1211x; 1.0341x over previous
"""Trainium2 Bass kernel: DiscreteEmbedding (rect-window embedding lookup).

Math (matches the jax reference up to fp16 table rounding):
    xs  = x * 2048;  y = xs + 0.5
    i_lo = floor(y)  (exact for non-boundary tokens)
    boundary tokens (y integer): out = avg of rows y-1, y -> combined table
Combined table TC (built on the HOST in fp16):
    TC[0:2048]  = T        (plain rows; TC[2048] = 0 zero row)
    TC[2048+k]  = (T[k-1]+T[k])/2 for k>=1   i.e. idx = i0 + 2048*b,
    b = (y integer), i0 = trunc(y).
Device: per-token SWDGE gather from TC, no device-side table prep.

Strategy notes:
  - fp16 halves all table/gather/store bytes; rel err ~2e-4.
  - Index math is sliced into 4 slabs of 2048 tokens so the first gather
    chunk dispatches as soon as slab 0 is done (SWDGE desc-gen at ~10ns/idx
    per queue-pair is the serial floor; start it early, keep 4 queues fed).
  - x loads split across both HWDGE rings (sync+scalar) to halve latency.
  - 8 uniform 1024-idx gather chunks: single num_idxs register (no $R WAR
    stall on the in-order GPSIMD engine), 12 SWDGE insts over 8 DMASW sem
    lanes (no gather-waits-gather ring serialization).
  - Gather output is position-ordered; stores alternate the two HWDGE
    rings; host un-permutes rows + casts fp16->f32 while un-sharding.
"""

import numpy as np

import concourse.mybir as mybir
import concourse.tile as tile
from concourse import bacc, bass_utils

N_CORES = 8
B, S = 32, 2048
V, D = 2048, 128
TOK = B * S                 # 65536 tokens total
TPC = TOK // N_CORES        # 8192 tokens per core
SPC = TPC // 16             # 512: free dim of the wrapped [16, 512] x layout
VEXT = 4224                 # TC rows (>= 2*V+1, multiple of 128)
NQ = 4                      # SWDGE queues
# gather chunks (num_idxs, queue): uniform 1024-idx chunks round-robin
CHUNKS = [(1024, 0), (1024, 1), (1024, 2), (1024, 3),
          (1024, 0), (1024, 1), (1024, 2), (1024, 3)]

F32 = mybir.dt.float32
F16 = mybir.dt.float16
I32 = mybir.dt.int32
I16 = mybir.dt.int16
OP = mybir.AluOpType


def build():
    nc = bacc.Bacc(
        "TRN2",
        target_bir_lowering=False,
        debug=False,
        num_devices=N_CORES,
        num_swdge_queues=NQ,
    )
    xr = nc.dram_tensor("xr", [128, SPC], F32, kind="ExternalInput")
    tc_tbl = nc.dram_tensor("tc_tbl", [VEXT, D], F16, kind="ExternalInput")
    out = nc.dram_tensor("out", [TPC, D], F16, kind="ExternalOutput")

    with tile.TileContext(nc) as tc:
        with tc.tile_pool(name="sb", bufs=1) as sb, tc.tile_pool(name="g", bufs=1) as gp:
            # ---- all gpsimd memsets + reg loads first ----
            zidx = sb.tile([128, 16], I16)
            nc.gpsimd.memset(zidx[:], 0)
            reg16 = nc.gpsimd.to_reg(16)
            nidx_regs = {
                n: nc.gpsimd.to_reg(n) for n in sorted({c[0] for c in CHUNKS})
            }
            # ---- single warm-up on queue 3: pays the Q7 library-load +
            # ring init without colliding with round-1 chunk dispatches
            # (q3 gets its first real chunk last).
            with tc.high_priority():
                wg = sb.tile([128, D], F16, tag="warm0")
                nc.gpsimd.dma_gather(
                    wg[:].rearrange("p (j d) -> p j d", d=D),
                    tc_tbl[:],
                    zidx[:, 0:1],
                    num_idxs=16,
                    num_idxs_reg=reg16,
                    elem_size=D,
                    single_packet=False,
                    queue_num=3,
                )

            # x halves on both HWDGE rings in parallel
            xt = sb.tile([128, SPC], F32)
            nc.sync.dma_start(out=xt[:, 0 : SPC // 2], in_=xr[:, 0 : SPC // 2])
            nc.scalar.dma_start(out=xt[:, SPC // 2 :], in_=xr[:, SPC // 2 :])

            # ---- index math: one full-width pass (the first SWDGE op is
            # gated on the end of the vector program, so ending the chain
            # sooner beats slab-chunking it) ----
            # y = x*2048 + 0.5 (exact in f32: x*2048 is a pow2 scale and y
            # is a multiple of 2^-13 below 2^11+1, so <= 24 bits)
            y = sb.tile([128, SPC], F32)
            nc.vector.tensor_scalar(y[:], xt[:], 2048.0, 0.5, op0=OP.mult, op1=OP.add)
            i0 = sb.tile([128, SPC], I32)
            nc.vector.tensor_copy(i0[:], y[:])       # RNE: floor or ceil
            f0 = sb.tile([128, SPC], F32)
            nc.vector.tensor_copy(f0[:], i0[:])
            gt = sb.tile([128, SPC], F32)
            nc.vector.tensor_tensor(gt[:], f0[:], y[:], op=OP.is_gt)
            bnd = sb.tile([128, SPC], F32)
            nc.vector.tensor_tensor(bnd[:], f0[:], y[:], op=OP.is_equal)
            # idx = floor(y) + 2048*b = f0 - gt + 2048*b
            u = sb.tile([128, SPC], F32)
            nc.vector.scalar_tensor_tensor(
                out=u[:], in0=bnd[:], scalar=float(V), in1=gt[:],
                op0=OP.mult, op1=OP.subtract,
            )
            idxf = sb.tile([128, SPC], F32)
            nc.vector.tensor_add(idxf[:], u[:], f0[:])
            idx16 = sb.tile([128, SPC], I16)
            nc.vector.tensor_copy(idx16[:], idxf[:])

            # ---- chunked gather + store ----
            out_v = out[:].rearrange("(p j) d -> p (j d)", p=128)
            pos = 0
            for ci, (nix, q) in enumerate(CHUNKS):
                jb = nix // 128
                g = gp.tile([128, jb * D], F16, tag=f"g{ci}")
                nc.gpsimd.dma_gather(
                    g[:].rearrange("p (j d) -> p j d", d=D),
                    tc_tbl[:],
                    idx16[:, pos // 16 : (pos + nix) // 16],
                    num_idxs=nix,
                    num_idxs_reg=nidx_regs[nix],
                    elem_size=D,
                    single_packet=False,
                    queue_num=q,
                )
                st_eng = nc.sync if ci % 2 == 0 else nc.scalar
                st_eng.dma_start(
                    out=out_v[:, (pos // 128) * D : ((pos + nix) // 128) * D], in_=g[:]
                )
                pos += nix
            assert pos == TPC
    nc.compile()
    return nc


_NC = None


def _row_perm():
    """out row r holds gather position i(r); position i handles token
    t(i) = (i%16)*512 + i//16 (x wrapped [16,512] across partitions)."""
    r = np.arange(TPC)
    p, j = r // 64, r % 64
    i = j * 128 + p
    return (i % 16) * SPC + i // 16  # token index held at row r


def _build_tc(t):
    """Host-side combined table, fp16: [T; 0 at 2048; avg-pairs at 2048+k]."""
    tc = np.zeros((VEXT, D), dtype=np.float32)
    tc[0:V] = t
    tz = np.vstack([t, np.zeros((1, D), np.float32)])
    # boundary y=k integer (k in 1..2048): avg of rows k-1, k at index 2048+k
    tc[V + 1 : V + 1 + V] = 0.5 * (tz[0:V] + tz[1 : V + 1])
    return tc.astype(np.float16)


def kernel(x, time_embedding):
    global _NC
    x = np.ascontiguousarray(np.asarray(x, dtype=np.float32))
    t = np.ascontiguousarray(np.asarray(time_embedding, dtype=np.float32))
    tc16 = _build_tc(t)
    xf = x.reshape(-1)
    in_maps = []
    for c in range(N_CORES):
        xc = xf[c * TPC : (c + 1) * TPC].reshape(16, SPC)
        in_maps.append({"xr": np.ascontiguousarray(np.tile(xc, (8, 1))), "tc_tbl": tc16})

    if _NC is None:
        _NC = build()
    res = bass_utils.run_bass_kernel_spmd(_NC, in_maps, core_ids=list(range(N_CORES)))
    global _LAST_RES
    _LAST_RES = res

    tkn = _row_perm()
    outs = []
    for c in range(N_CORES):
        oc = np.asarray(res.results[c]["out"]).astype(np.float32)
        full = np.empty_like(oc)
        full[tkn] = oc
        outs.append(full)
    return np.concatenate(outs, axis=0).reshape(B, S, D)
